# revision 11
# baseline (speedup 1.0000x reference)
"""Sliding-window + sink attention (GQA, RoPE, RMSNorm) on 8 TRN2 NeuronCores.

Sharding: sequence-parallel. Each core owns 512 query rows and the
1028-key halo (4 sink keys + 1024-row window) those queries can attend
to. No collectives: each core computes its full output rows including
the Wo projection; the host concatenates the 8 row-blocks.

Layout: everything kept "transposed" (feature dim on partitions) so no
on-device transposes are needed. The 80-dim head vectors live on 128
partitions: RoPE-even dims at partitions 0..39, odd dims at 64..103,
zeros elsewhere (partition starts must be 32-aligned; the zero rows are
free in the K=128 contraction). The softmax denominator is folded into
the PV matmul as a ones-column of V at aligned output partition 96; the
[1,512] -> [80,512] reciprocal broadcast is a K=1 outer-product matmul.
"""

import math

import numpy as np
import ml_dtypes

import concourse.bass as bass
import concourse.bacc as bacc
import concourse.tile as tile
from concourse import mybir
from concourse.bass_utils import run_bass_kernel_spmd

BF16 = ml_dtypes.bfloat16

HEAD_DIM = 80
N_HEADS = 8
N_KV_HEADS = 2
N_REP = N_HEADS // N_KV_HEADS
EPS = 1e-5
WINDOW = 512
SINK = 4
SEQ = 4096
HIDDEN = 640
N_CORES = 8
SQ = SEQ // N_CORES          # queries per core
KV = SINK + 2 * WINDOW       # 1028 keys per core (4 sink + 1024 window)
NJT = 9                      # ceil(1028/128) key tiles, last has 4 rows
KT_PAD = NJT * 128           # 1152
SCALE = 1.0 / math.sqrt(HEAD_DIM)
DP = 128                     # padded head dim on partitions
VH = 97                      # per-kv V block: 80 V cols + 16 pad + ones col
VW = 2 * VH                  # 194
DEN = 96                     # aligned partition of the denominator row

_CACHE = {}


def _jw(jt):
    return min(128, KV - jt * 128)


def build_nc():
    nc = bacc.Bacc(None, target_bir_lowering=False, debug=False)
    f32 = mybir.dt.float32
    bf16 = mybir.dt.bfloat16

    xqT = nc.declare_dram_parameter("xqT", [HIDDEN, SQ], bf16, isOutput=False)
    xkT = nc.declare_dram_parameter("xkT", [HIDDEN, KV], bf16, isOutput=False)
    Wq = nc.declare_dram_parameter("Wq", [HIDDEN, N_HEADS * DP], bf16, isOutput=False)
    Wk = nc.declare_dram_parameter("Wk", [HIDDEN, N_KV_HEADS * DP], bf16, isOutput=False)
    Wv = nc.declare_dram_parameter("Wv", [HIDDEN, VW], bf16, isOutput=False)
    Wo = nc.declare_dram_parameter("Wo", [HIDDEN, HIDDEN], bf16, isOutput=False)
    mT = nc.declare_dram_parameter("mT", [KT_PAD, SQ], bf16, isOutput=False)
    cq = nc.declare_dram_parameter("cq", [DP, SQ], bf16, isOutput=False)
    sq = nc.declare_dram_parameter("sq", [DP, SQ], bf16, isOutput=False)
    ck = nc.declare_dram_parameter("ck", [DP, KV], bf16, isOutput=False)
    sk = nc.declare_dram_parameter("sk", [DP, KV], bf16, isOutput=False)
    qnw = nc.declare_dram_parameter("qnw", [DP, 1], f32, isOutput=False)
    knw = nc.declare_dram_parameter("knw", [DP, 1], f32, isOutput=False)
    out = nc.declare_dram_parameter("out", [SQ, HIDDEN], f32, isOutput=True)

    with tile.TileContext(nc) as tc:
        with (
            tc.tile_pool(name="w", bufs=1) as wp,
            tc.tile_pool(name="act", bufs=1) as ap,
            tc.tile_pool(name="tmp", bufs=1) as tp,
            tc.tile_pool(name="sm", bufs=2) as smp,
            tc.tile_pool(name="pt", bufs=2) as ptp,
            tc.tile_pool(name="ps", bufs=2, space="PSUM") as ps,
        ):
            # ---- load everything to SBUF ----
            xq_sb = wp.tile([128, 5, SQ], bf16)
            nc.sync.dma_start(xq_sb[:], xqT.rearrange("(t p) q -> p t q", p=128))
            xk_sb = wp.tile([128, 5, KV], bf16)
            nc.sync.dma_start(xk_sb[:], xkT.rearrange("(t p) q -> p t q", p=128))
            wq_sb = wp.tile([128, 5, N_HEADS * DP], bf16)
            nc.sync.dma_start(wq_sb[:], Wq.rearrange("(t p) n -> p t n", p=128))
            wk_sb = wp.tile([128, 5, N_KV_HEADS * DP], bf16)
            nc.sync.dma_start(wk_sb[:], Wk.rearrange("(t p) n -> p t n", p=128))
            wv_sb = wp.tile([128, 5, VW], bf16)
            nc.sync.dma_start(wv_sb[:], Wv.rearrange("(t p) n -> p t n", p=128))
            wo_sb = wp.tile([HEAD_DIM, N_HEADS, HIDDEN], bf16)
            nc.sync.dma_start(wo_sb[:], Wo.rearrange("(h d) n -> d h n", d=HEAD_DIM))
            m_sb = wp.tile([128, NJT, SQ], bf16)
            nc.sync.dma_start(m_sb[:], mT.rearrange("(t p) q -> p t q", p=128))
            cq_sb = wp.tile([DP, SQ], bf16)
            nc.sync.dma_start(cq_sb[:], cq[:])
            sq_sb = wp.tile([DP, SQ], bf16)
            nc.sync.dma_start(sq_sb[:], sq[:])
            ck_sb = wp.tile([DP, KV], bf16)
            nc.sync.dma_start(ck_sb[:], ck[:])
            sk_sb = wp.tile([DP, KV], bf16)
            nc.sync.dma_start(sk_sb[:], sk[:])
            qnw_sb = wp.tile([DP, 1], f32)
            nc.sync.dma_start(qnw_sb[:], qnw[:])
            knw_sb = wp.tile([DP, 1], f32)
            nc.sync.dma_start(knw_sb[:], knw[:])

            ones_c = wp.tile([DP, 1], bf16)    # sumsq lhsT [K=128, M=1]
            nc.vector.memset(ones_c[:], 1.0)
            ones_r = wp.tile([1, DP], bf16)    # bcast lhsT [K=1, M=128]
            nc.vector.memset(ones_r[:], 1.0)
            eps_sb = wp.tile([1, 1], f32)
            nc.vector.memset(eps_sb[:], EPS)

            # ---- projections ----
            qt_f = ap.tile([DP, N_HEADS, SQ], f32)
            for h in range(N_HEADS):
                p = ps.tile([128, SQ], f32, tag="mm")
                for kt in range(5):
                    nc.tensor.matmul(
                        p[:], wq_sb[:, kt, h * DP:(h + 1) * DP], xq_sb[:, kt, :],
                        start=(kt == 0), stop=(kt == 4))
                nc.vector.tensor_copy(qt_f[:, h, :], p[:])

            kt_f = ap.tile([DP, N_KV_HEADS, KV], f32)
            for kvh in range(N_KV_HEADS):
                for c0, cw in ((0, 512), (512, 512), (1024, 4)):
                    p = ps.tile([128, SQ], f32, tag="mm")
                    for kt in range(5):
                        nc.tensor.matmul(
                            p[:, :cw], wk_sb[:, kt, kvh * DP:(kvh + 1) * DP],
                            xk_sb[:, kt, c0:c0 + cw],
                            start=(kt == 0), stop=(kt == 4))
                    nc.vector.tensor_copy(kt_f[:, kvh, c0:c0 + cw], p[:, :cw])

            v_sb = ap.tile([128, NJT, VW], bf16)
            for jt in range(NJT):
                jw = _jw(jt)
                p = ps.tile([128, SQ], f32, tag="mm")
                for kt in range(5):
                    nc.tensor.matmul(
                        p[:jw, :VW], xk_sb[:, kt, jt * 128:jt * 128 + jw],
                        wv_sb[:, kt, :],
                        start=(kt == 0), stop=(kt == 4))
                nc.vector.tensor_copy(v_sb[:jw, jt, :], p[:jw, :VW])
                nc.vector.memset(v_sb[:jw, jt, DEN:DEN + 1], 1.0)
                nc.vector.memset(v_sb[:jw, jt, VH + DEN:VH + DEN + 1], 1.0)

            # ---- RMSNorm (feature dim = partitions -> ones-matmul stats) ----
            def rmsnorm(src_f32, nh, ncols, w_sb, tag):
                # returns bf16 = src * w * rsqrt(mean_80(src^2)+eps); zero pad
                # rows stay zero because w is zero there.
                sq_t = tp.tile([DP, nh, ncols], bf16, tag=tag + "sq")
                nc.vector.tensor_mul(sq_t[:], src_f32[:], src_f32[:])
                rbf = tp.tile([1, nh, ncols], bf16, tag=tag + "rb")
                dst = tp.tile([DP, nh, ncols], bf16, tag=tag + "o")
                for h in range(nh):
                    for c0 in range(0, ncols, 512):
                        cw = min(512, ncols - c0)
                        p = ps.tile([128, SQ], f32, tag="mm")
                        nc.tensor.matmul(p[:1, :cw], ones_c[:], sq_t[:, h, c0:c0 + cw],
                                         start=True, stop=True)
                        sc = smp.tile([1, 512], f32, tag="nsc")
                        nc.scalar.activation(sc[:, :cw], p[:1, :cw],
                                             mybir.ActivationFunctionType.Sqrt,
                                             scale=1.0 / HEAD_DIM, bias=eps_sb[:])
                        with nc.allow_low_precision("rstd fits bf16; 2e-2 gate"):
                            nc.vector.reciprocal(rbf[:, h, c0:c0 + cw], sc[:, :cw])
                for h in range(nh):
                    for c0 in range(0, ncols, 512):
                        cw = min(512, ncols - c0)
                        p = ps.tile([128, SQ], f32, tag="mm")
                        nc.tensor.matmul(p[:, :cw], ones_r[:], rbf[:, h, c0:c0 + cw],
                                         start=True, stop=True)
                        nc.vector.scalar_tensor_tensor(
                            dst[:, h, c0:c0 + cw], p[:, :cw], w_sb[:],
                            src_f32[:, h, c0:c0 + cw],
                            op0=mybir.AluOpType.mult, op1=mybir.AluOpType.mult)
                return dst

            qt_r = rmsnorm(qt_f, N_HEADS, SQ, qnw_sb, "nq")
            kt_r = rmsnorm(kt_f, N_KV_HEADS, KV, knw_sb, "nk")

            # ---- RoPE in place: x = x*[c;c] + swap(x)*[-s;s] (sign in s_sb).
            # swap copies whole 64-partition halves so pad rows stay zero. ----
            def rope(v, nh, ncols, c_sb, s_sb):
                for h in range(nh):
                    sw = smp.tile([DP, KV], bf16, tag="rsw")
                    nc.vector.tensor_copy(sw[0:64, :ncols], v[64:128, h, :])
                    nc.vector.tensor_copy(sw[64:128, :ncols], v[0:64, h, :])
                    nc.vector.tensor_mul(sw[:, :ncols], sw[:, :ncols], s_sb[:])
                    nc.vector.tensor_mul(v[:, h, :], v[:, h, :], c_sb[:])
                    nc.vector.tensor_add(v[:, h, :], v[:, h, :], sw[:, :ncols])

            rope(qt_r, N_HEADS, SQ, cq_sb, sq_sb)
            rope(kt_r, N_KV_HEADS, KV, ck_sb, sk_sb)

            # ---- attention per head ----
            otn = ap.tile([HEAD_DIM, N_HEADS, SQ], bf16)
            for h in range(N_HEADS):
                kvh = h // N_REP
                pt = ptp.tile([128, NJT, SQ], bf16, tag="pt")
                for jt in range(NJT):
                    jw = _jw(jt)
                    st = ps.tile([128, SQ], f32, tag="st")
                    nc.tensor.matmul(
                        st[:jw, :], kt_r[:, kvh, jt * 128:jt * 128 + jw],
                        qt_r[:, h, :], start=True, stop=True)
                    nc.scalar.activation(pt[:jw, jt, :], st[:jw, :],
                                         mybir.ActivationFunctionType.Exp,
                                         scale=SCALE)
                    nc.vector.tensor_mul(pt[:jw, jt, :], pt[:jw, jt, :],
                                         m_sb[:jw, jt, :])
                ot = ps.tile([DEN + 1, SQ], f32, tag="ot")
                for jt in range(NJT):
                    jw = _jw(jt)
                    nc.tensor.matmul(
                        ot[:], v_sb[:jw, jt, kvh * VH:kvh * VH + DEN + 1],
                        pt[:jw, jt, :], start=(jt == 0), stop=(jt == NJT - 1))
                rden_bf = smp.tile([1, SQ], bf16, tag="rdbf")
                with nc.allow_low_precision("softmax denom fits bf16; 2e-2 gate"):
                    nc.vector.reciprocal(rden_bf[:], ot[DEN:DEN + 1, :])
                rb = ps.tile([128, SQ], f32, tag="mm")
                nc.tensor.matmul(rb[:HEAD_DIM, :], ones_r[:, :HEAD_DIM], rden_bf[:],
                                 start=True, stop=True)
                rb_sb = smp.tile([HEAD_DIM, SQ], bf16, tag="rbsb")
                nc.vector.tensor_copy(rb_sb[:], rb[:HEAD_DIM, :])
                nc.vector.tensor_mul(otn[:, h, :], ot[0:HEAD_DIM, :], rb_sb[:])

            # ---- output projection ----
            y_sb = ap.tile([128, 4, HIDDEN], f32)
            for qt in range(4):
                for n0, nw in ((0, 512), (512, 128)):
                    yp = ps.tile([128, SQ], f32, tag="mm")
                    for h in range(N_HEADS):
                        nc.tensor.matmul(
                            yp[:, :nw], otn[:, h, qt * 128:(qt + 1) * 128],
                            wo_sb[:, h, n0:n0 + nw],
                            start=(h == 0), stop=(h == N_HEADS - 1))
                    nc.vector.tensor_copy(y_sb[:, qt, n0:n0 + nw], yp[:, :nw])
            nc.sync.dma_start(out.rearrange("(t p) n -> p t n", p=128), y_sb[:])

    nc.finalize()
    return nc


# permutation of one head's 80 dims into the 128-partition padded layout
_SRC = np.concatenate([np.arange(0, HEAD_DIM, 2),      # -> partitions 0..39
                       np.arange(1, HEAD_DIM, 2)])     # -> partitions 64..103
_DSTP = np.concatenate([np.arange(40), np.arange(64, 104)])


def _pad_head_cols(W, nh):
    """[HIDDEN, nh*80] -> [HIDDEN, nh*128] with evens at 0..39, odds 64..103."""
    out = np.zeros((W.shape[0], nh * DP), np.float32)
    for h in range(nh):
        out[:, h * DP + _DSTP] = W[:, h * HEAD_DIM + _SRC]
    return out


def _pad_rope_rows(tab, sign=False):
    """[40, n] cos/sin table -> [128, n] padded; sign=True gives [-s; +s]."""
    out = np.zeros((DP, tab.shape[1]), np.float32)
    out[0:40] = -tab if sign else tab
    out[64:104] = tab
    return out


def _prep_core(c, xT, cosT, sinT, shared):
    qpos = c * SQ + np.arange(SQ)

    xq = np.ascontiguousarray(xT[:, qpos].astype(BF16))

    # window rows r=4..1027 cover j = c*SQ-WINDOW .. c*SQ+WINDOW-1
    kpos = np.empty(KV, np.int64)
    kpos[:SINK] = np.arange(SINK)
    kpos[SINK:] = c * SQ - WINDOW + np.arange(2 * WINDOW)
    valid = (kpos >= 0) & (kpos < SEQ)
    kcl = np.clip(kpos, 0, SEQ - 1)
    xk = xT[:, kcl].copy()
    xk[:, ~valid] = 0.0
    xk = np.ascontiguousarray(xk.astype(BF16))

    # mask [KT_PAD, SQ]: sink rows authoritative for j<4; window rows j>=4 only
    j = kpos[:, None]
    i = qpos[None, :]
    allow = np.zeros((KT_PAD, SQ), np.float32)
    allow[:SINK] = (j[:SINK] <= i)
    allow[SINK:KV] = ((j[SINK:] >= SINK) & valid[SINK:, None]
                      & (j[SINK:] <= i) & (j[SINK:] >= i - (WINDOW - 1)))
    mT = allow.astype(BF16)

    d = dict(
        xqT=xq, xkT=xk, mT=mT,
        cq=_pad_rope_rows(cosT[:, qpos]).astype(BF16),
        sq=_pad_rope_rows(sinT[:, qpos], sign=True).astype(BF16),
        ck=_pad_rope_rows(cosT[:, kcl]).astype(BF16),
        sk=_pad_rope_rows(sinT[:, kcl], sign=True).astype(BF16),
    )
    d.update(shared)
    return d


def _prep_shared(Wq, Wk, Wv, Wo, q_norm_w, k_norm_w):
    Wv_a = np.zeros((HIDDEN, VW), np.float32)
    Wv_a[:, 0:80] = Wv[:, 0:80]
    Wv_a[:, VH:VH + 80] = Wv[:, 80:160]
    qnw = np.zeros((DP, 1), np.float32)
    qnw[_DSTP, 0] = q_norm_w[_SRC]
    knw = np.zeros((DP, 1), np.float32)
    knw[_DSTP, 0] = k_norm_w[_SRC]
    return dict(
        Wq=np.ascontiguousarray(_pad_head_cols(Wq, N_HEADS).astype(BF16)),
        Wk=np.ascontiguousarray(_pad_head_cols(Wk, N_KV_HEADS).astype(BF16)),
        Wv=np.ascontiguousarray(Wv_a.astype(BF16)),
        Wo=np.ascontiguousarray(Wo.astype(BF16)),
        qnw=qnw, knw=knw,
    )


def make_in_maps(x, cos, sin, Wq, Wk, Wv, Wo, q_norm_w, k_norm_w):
    xT = np.ascontiguousarray(x[0].T.astype(np.float32))
    cosT = np.ascontiguousarray(cos.T.astype(np.float32))
    sinT = np.ascontiguousarray(sin.T.astype(np.float32))
    shared = _prep_shared(Wq, Wk, Wv, Wo, q_norm_w, k_norm_w)
    return [_prep_core(c, xT, cosT, sinT, shared) for c in range(N_CORES)]


def kernel(x, cos, sin, Wq, Wk, Wv, Wo, q_norm_w, k_norm_w):
    if "nc" not in _CACHE:
        _CACHE["nc"] = build_nc()
    nc = _CACHE["nc"]
    in_maps = make_in_maps(x, cos, sin, Wq, Wk, Wv, Wo, q_norm_w, k_norm_w)
    _CACHE["in_maps"] = in_maps
    res = run_bass_kernel_spmd(nc, in_maps, core_ids=list(range(N_CORES)))
    blocks = [np.asarray(res.results[c]["out"]) for c in range(N_CORES)]
    return np.concatenate(blocks, axis=0)[None].astype(np.float32)
